# revision 30
# baseline (speedup 1.0000x reference)
"""AttentionBlock (GroupNorm + single-head spatial attention + SE gate + residual)
Trainium2 Bass/Tile kernel, data-parallel over batch across 8 NeuronCores.

Full shapes: x [32, 256, 32, 32] f32 -> out [32, 256, 32, 32] f32.
Per core: 4 samples. Per sample (C=256, N=1024), zero-bias fast path:
  xn = GroupNorm(x) (32 groups)            [C, N]  (fp8e4)
  u  = G @ xn, G = 256*Wk^T Wq (host fp8)  [C, N]  (fp8e4)  <- no separate q,k!
  esT = exp((xn^T u) / (16*256))           [N, N]  ([j, i] layout, fp8e4)
  vT = xn^T @ WvT                          [N, C]  (fp8e4)
  sums = ones @ esT  (accum over j)        [128, N]
  r = 1/sums (reciprocal_approx_fast)      [128, N]
  xat = (vT^T @ esT) * r                   [C, N]  (fp8e4)
  y = Wp @ xat                             [C, N]
  out = x + y * gate[c]                    (gate = SE sigmoid / 1024)

All attention matmuls are fp8e4 MatmulPerfMode.DoubleRow (two 128-deep
k-tiles per instruction, 2x bf16 rate: one 512-col matmul per ~216ns).
Host weights pre-scaled by 32 (Wv, Wp) / 256 (G) to sit in e4m3 range;
compensations fold into the exp scale and the SE gate (1/1024) for free.

The ACT engine is the pipeline limiter (8 [128,1024] exps + 2 u-evacs
per sample ~= 11.4us). Everything else is kept off ACT: vt/av/proj
evacuations on DVE, xn on GpSimd, and ALL GroupNorm/SE/stats work for
the 4 samples is hoisted into the DMA/warm-up head where DVE idles.
rstd = 1/sqrt(var+eps) is computed with a tiny Newton iteration on DVE
(GN var ~1) -- no Ln/Sqrt ACT table reloads mid-kernel.

If qkv biases are nonzero (not the case for this model's inputs) a
general program variant with explicit q,k evacuation is built instead.
"""

import numpy as np
import ml_dtypes

B, C, HW, N = 32, 256, 32, 1024
NCORES = 8
BL = B // NCORES          # samples per core
GROUPS = 32
GSIZE = C // GROUPS       # 8 channels per group
EPS = 1e-5
CT = 2                    # channel partition tiles (256 = 2*128)
P = 128
WS = 32.0                 # host-side fp8 weight scale (wv, wp)
GS = 256.0                # host-side fp8 scale for G = Wk^T Wq

_CACHE = {}


def _build_program(flags):
    has_bqk, has_bv, has_bp = flags
    import concourse.bacc as bacc
    import concourse.mybir as mybir
    import concourse.tile as tile

    f32 = mybir.dt.float32
    fp8 = mybir.dt.float8e4
    bf16 = mybir.dt.bfloat16
    AX = mybir.AxisListType.X
    AF = mybir.ActivationFunctionType
    ALU = mybir.AluOpType
    DR = mybir.MatmulPerfMode.DoubleRow

    nc = bacc.Bacc()

    # ---- DRAM I/O ----
    x_d = nc.dram_tensor("x", [BL, C, N], f32, kind="ExternalInput")
    out_d = nc.dram_tensor("out", [BL, C, N], f32, kind="ExternalOutput")
    # gt = (Wq^T Wk)*GS partition-tiled (zero-bias path); wqk kept for the
    # general biased path
    gt_d = nc.dram_tensor("gt", [P, 2, C], fp8, kind="ExternalInput")
    wqk_d = nc.dram_tensor("wqk", [P, 2, 512], fp8, kind="ExternalInput")
    wv_d = nc.dram_tensor("wv", [P, 2, C], fp8, kind="ExternalInput")
    wp_d = nc.dram_tensor("wp", [P, 2, C], fp8, kind="ExternalInput")
    w1_d = nc.dram_tensor("w1", [P, 2, 64], f32, kind="ExternalInput")
    w2_d = nc.dram_tensor("w2", [64, C], f32, kind="ExternalInput")
    gamma_d = nc.dram_tensor("gamma", [P, 2], f32, kind="ExternalInput")
    beta_d = nc.dram_tensor("beta", [P, 2], f32, kind="ExternalInput")
    bqk_d = nc.dram_tensor("bqk", [P, 4], f32, kind="ExternalInput")
    bv_d = nc.dram_tensor("bv", [P, 2], f32, kind="ExternalInput")
    bp_d = nc.dram_tensor("bp", [P, 2], f32, kind="ExternalInput")
    b1_d = nc.dram_tensor("b1", [64, 1], f32, kind="ExternalInput")
    b2_d = nc.dram_tensor("b2", [P, 2], f32, kind="ExternalInput")
    gm_d = nc.dram_tensor("gm", [P, 16], f32, kind="ExternalInput")
    gmt_d = nc.dram_tensor("gmt", [16, P], f32, kind="ExternalInput")

    with tile.TileContext(nc) as tc:
        with (
            tc.tile_pool(name="persist", bufs=1) as persist,
            tc.tile_pool(name="uu", bufs=2) as u_pool,
            tc.tile_pool(name="vt", bufs=2) as vt_pool,
            tc.tile_pool(name="es", bufs=2) as es_pool,
            tc.tile_pool(name="xat", bufs=2) as xat_pool,
            tc.tile_pool(name="rr", bufs=2) as r_pool,
            tc.tile_pool(name="junk", bufs=2) as junk_pool,
            tc.tile_pool(name="outp", bufs=3) as out_pool,
            tc.tile_pool(name="psb", bufs=3, space="PSUM") as psum_big,
            tc.tile_pool(name="pss", bufs=2, space="PSUM") as psum_small,
        ):
            # ---- DMA prologue: x slices first (sample 0's stats are the
            # critical path), then consts/weights in first-use order.
            # warm-up constants first: the DVE memset must head the DVE
            # queue (no DMA dependency) so the PE warm matmuls start ~1.5us
            warm_sb = persist.tile([P, 512], bf16)
            nc.vector.memset(warm_sb, 1.0)
            ones_sb = persist.tile([P, 2, P], fp8)
            nc.gpsimd.memset(ones_sb, 1.0)

            x_sb = persist.tile([P, CT, BL, N], f32)

            def load_x(b):
                for ct in range(CT):
                    nc.sync.dma_start(out=x_sb[:, ct, b],
                                      in_=x_d[b, ct * P:(ct + 1) * P, :])

            load_x(0)
            gm_sb = persist.tile([P, 16], f32)
            nc.sync.dma_start(out=gm_sb, in_=gm_d[:, :])
            gmt_sb = persist.tile([16, P], f32)
            nc.sync.dma_start(out=gmt_sb, in_=gmt_d[:, :])
            load_x(1)
            gamma_sb = persist.tile([P, 2], f32)
            nc.sync.dma_start(out=gamma_sb, in_=gamma_d[:, :])
            beta_sb = persist.tile([P, 2], f32)
            nc.sync.dma_start(out=beta_sb, in_=beta_d[:, :])
            if has_bqk:
                wqk_sb = persist.tile([P, 2, 512], fp8)
                nc.sync.dma_start(out=wqk_sb, in_=wqk_d[:, :, :])
                bqk_sb = persist.tile([P, 4], f32)
                nc.sync.dma_start(out=bqk_sb, in_=bqk_d[:, :])
            else:
                gt_sb = persist.tile([P, 2, C], fp8)
                nc.sync.dma_start(out=gt_sb, in_=gt_d[:, :, :])
            wv_sb = persist.tile([P, 2, C], fp8)
            nc.sync.dma_start(out=wv_sb, in_=wv_d[:, :, :])
            load_x(2)
            load_x(3)
            bv_sb = persist.tile([P, 2], f32)
            nc.sync.dma_start(out=bv_sb, in_=bv_d[:, :])
            bp_sb = persist.tile([P, 2], f32)
            nc.sync.dma_start(out=bp_sb, in_=bp_d[:, :])
            b1_sb = persist.tile([64, 1], f32)
            nc.sync.dma_start(out=b1_sb, in_=b1_d[:, :])
            b2_sb = persist.tile([P, 2], f32)
            nc.sync.dma_start(out=b2_sb, in_=b2_d[:, :])
            w1_sb = persist.tile([P, 2, 64], f32)
            nc.sync.dma_start(out=w1_sb, in_=w1_d[:, :, :])
            w2_sb = persist.tile([64, C], f32)
            nc.sync.dma_start(out=w2_sb, in_=w2_d[:, :])
            wp_sb = persist.tile([P, 2, C], fp8)
            nc.sync.dma_start(out=wp_sb, in_=wp_d[:, :, :])

            nb2_sb = persist.tile([P, 2], f32)

            # ---- persistent intermediates ----
            bn_sb = persist.tile([P, CT, BL, 12], f32)  # bn_stats (2 seg x 6)
            msum_c = persist.tile([P, CT, BL], f32)     # channel sum / 256
            s2_c = persist.tile([P, CT, BL], f32)       # channel sum of x^2
            cv_c = persist.tile([P, CT, BL], f32)       # sum of count*var
            a_sb = persist.tile([P, CT, BL], f32)       # per-channel scale
            bb_sb = persist.tile([P, CT, BL], f32)      # per-channel offset
            xn_sb = persist.tile([P, CT, BL, N], fp8)
            gate_sb = persist.tile([P, CT, BL], f32)    # sigmoid/1024
            bpg_sb = persist.tile([P, CT, BL], f32)     # bp*sigmoid (bias case)
            h1_sb = persist.tile([64, BL], f32)
            qk_tiles = [None] * BL

            def emit_stats(b):
                # bn_stats per 512-elem segment gives count/mean/count*var
                # for even/odd interleaves in one DVE read of x.
                # sum_c = 256*sum(means); sumsq_c = sum(cv) + 256*sum(mean^2)
                for ct in range(CT):
                    for seg in range(2):
                        nc.vector.bn_stats(
                            out=bn_sb[:, ct, b, seg * 6:(seg + 1) * 6],
                            in_=x_sb[:, ct, b, seg * 512:(seg + 1) * 512])
                for ct in range(CT):
                    means = bn_sb[:, ct, b, 1::3]   # [P, 4] stride 3
                    cvs = bn_sb[:, ct, b, 2::3]     # [P, 4]
                    nc.vector.reduce_sum(
                        out=msum_c[:, ct, b:b + 1], in_=means, axis=AX)
                    nc.vector.reduce_sum(
                        out=cv_c[:, ct, b:b + 1], in_=cvs, axis=AX)
                    jt = junk_pool.tile([P, 4], f32, tag="junk4")
                    msq = junk_pool.tile([P, 1], f32, tag="junk1")
                    nc.vector.affine_mul_reduce(
                        out=jt, accum_out=msq, in0=means, in1=means,
                        scale=1.0, bias=0.0)
                    nc.vector.affine_then_add(
                        out=s2_c[:, ct, b:b + 1], in0=msq,
                        in1=cv_c[:, ct, b:b + 1], scale=256.0, bias=0.0)

            def emit_gn(b, xn_engines):
                # per-sample GroupNorm coefficients (a, bb) + xn write.
                # msum is channel_sum/256: group mean = gm@msum * 256/8192
                ps_g = psum_small.tile([16, 4], f32, tag="pss")
                for ct in range(CT):
                    nc.tensor.matmul(ps_g[:, ct:ct + 1], gm_sb,
                                     msum_c[:, ct, b:b + 1],
                                     start=True, stop=True)
                    nc.tensor.matmul(ps_g[:, 2 + ct:3 + ct], gm_sb,
                                     s2_c[:, ct, b:b + 1],
                                     start=True, stop=True)
                nmean = persist.tile([16, 2], f32)
                nc.vector.tensor_scalar_mul(nmean, ps_g[:, 0:2],
                                            -256.0 / (GSIZE * N))
                var = persist.tile([16, 2], f32)
                nc.vector.tensor_scalar_mul(var, ps_g[:, 2:4],
                                            1.0 / (GSIZE * N))
                msq = persist.tile([16, 2], f32)
                nc.vector.tensor_mul(msq, nmean, nmean)
                nc.vector.tensor_sub(var, var, msq)
                # rstd = 1/sqrt(var+eps) via Newton on tiny [16,2] DVE ops:
                # GN var is ~1 by construction so z0 = 1.5 - (var+eps)/2
                # converges quadratically; two iterations reach ~1e-5.
                # Avoids Ln/Sqrt on ACT (each costs a 1.28us table reload).
                vpe = persist.tile([16, 2], f32)
                nc.vector.tensor_scalar_add(vpe, var, EPS)
                rsm = persist.tile([16, 4], f32)
                z = rsm[:, 0:2]
                nc.vector.tensor_scalar(out=z, in0=vpe, scalar1=-0.5,
                                        scalar2=1.5, op0=ALU.mult, op1=ALU.add)
                zt = persist.tile([16, 2], f32)
                for _ in range(2):
                    nc.vector.tensor_mul(zt, z, z)
                    nc.vector.tensor_mul(zt, zt, vpe)
                    nc.vector.tensor_scalar(out=zt, in0=zt, scalar1=-0.5,
                                            scalar2=1.5, op0=ALU.mult,
                                            op1=ALU.add)
                    nc.vector.tensor_mul(z, z, zt)
                nc.vector.tensor_mul(rsm[:, 2:4], nmean, z)
                ps_bc = psum_small.tile([P, 4], f32, tag="pss")
                nc.tensor.matmul(ps_bc, gmt_sb, rsm, start=True, stop=True)
                for ct in range(CT):
                    nc.vector.tensor_scalar_mul(
                        a_sb[:, ct, b:b + 1], ps_bc[:, ct:ct + 1],
                        gamma_sb[:, ct:ct + 1])
                    nc.vector.tensor_scalar(
                        out=bb_sb[:, ct, b:b + 1], in0=ps_bc[:, 2 + ct:3 + ct],
                        scalar1=gamma_sb[:, ct:ct + 1],
                        scalar2=beta_sb[:, ct:ct + 1],
                        op0=ALU.mult, op1=ALU.add)
                for ct in range(CT):
                    eng = xn_engines[ct]
                    if eng == "act":
                        nc.scalar.activation(
                            out=xn_sb[:, ct, b], in_=x_sb[:, ct, b],
                            func=AF.Identity,
                            bias=bb_sb[:, ct, b:b + 1],
                            scale=a_sb[:, ct, b:b + 1])
                    elif eng == "dve":
                        nc.vector.tensor_scalar(
                            out=xn_sb[:, ct, b], in0=x_sb[:, ct, b],
                            scalar1=a_sb[:, ct, b:b + 1],
                            scalar2=bb_sb[:, ct, b:b + 1],
                            op0=ALU.mult, op1=ALU.add)
                    else:
                        nc.gpsimd.tensor_scalar(
                            out=xn_sb[:, ct, b], in0=x_sb[:, ct, b],
                            scalar1=a_sb[:, ct, b:b + 1],
                            scalar2=bb_sb[:, ct, b:b + 1],
                            op0=ALU.mult, op1=ALU.add)

            def emit_se(b):
                # sigmoid(z)/1024 = 1/(1024*(1+exp(-z))): stays in exp table
                # and folds the fp8 weight-scale compensation in for free.
                ps_h1 = psum_small.tile([64, 1], f32, tag="pss")
                for ct in range(CT):
                    nc.tensor.matmul(ps_h1, w1_sb[:, ct],
                                     msum_c[:, ct, b:b + 1],
                                     start=(ct == 0), stop=(ct == 1))
                # pooled = msum*256/1024 -> relu scale 0.25
                nc.scalar.activation(out=h1_sb[:, b:b + 1], in_=ps_h1,
                                     func=AF.Relu, bias=b1_sb[:, 0:1],
                                     scale=0.25)
                for ot in range(CT):
                    ps_gate = psum_small.tile([P, 1], f32, tag="pss")
                    nc.tensor.matmul(ps_gate, w2_sb[:, ot * P:(ot + 1) * P],
                                     h1_sb[:, b:b + 1], start=True, stop=True)
                    eg = persist.tile([P, 1], f32)
                    nc.scalar.activation(out=eg, in_=ps_gate, func=AF.Exp,
                                         scale=-1.0, bias=nb2_sb[:, ot:ot + 1])
                    nc.vector.tensor_scalar(
                        out=eg, in0=eg, scalar1=1024.0, scalar2=1024.0,
                        op0=ALU.mult, op1=ALU.add)
                    nc.vector.reciprocal(gate_sb[:, ot, b:b + 1], eg)
                    if has_bp:
                        # bp_sb holds 1024*bp -> bpg = bp*sigmoid
                        nc.vector.tensor_scalar_mul(bpg_sb[:, ot, b:b + 1],
                                                    gate_sb[:, ot, b:b + 1],
                                                    bp_sb[:, ot:ot + 1])

            def emit_u(b, both_act=False):
                # zero-bias path: u = (G/GS) @ xn so that S = xn^T u.
                # Two fp8 tiles replace four (q0,q1,k0,k1) evacuations;
                # copies split ACT/DVE to balance the two evac engines.
                u_sb = u_pool.tile([P, 2, N], fp8, tag="uu")
                qk_tiles[b] = u_sb
                for at in range(CT):
                    ps_u = psum_big.tile([P, N], f32, tag="psb")
                    for ns in range(2):
                        nc.tensor.matmul(
                            ps_u[:, ns * 512:(ns + 1) * 512],
                            gt_sb[:, :, at * P:(at + 1) * P],
                            xn_sb[:, :, b, ns * 512:(ns + 1) * 512],
                            start=True, stop=True, perf_mode=DR)
                    if at == 0 or both_act:
                        nc.scalar.copy(out=u_sb[:, at], in_=ps_u)
                    else:
                        nc.vector.tensor_copy(out=u_sb[:, at], in_=ps_u)

            def emit_qk_biased(b):
                # general path with nonzero qkv bias: q, k : [c, n] fp8
                qk_sb = u_pool.tile([P, 4, N], fp8, tag="uu")
                qk_tiles[b] = qk_sb
                for m in (2, 3, 0, 1):
                    ps_qk = psum_big.tile([P, N], f32, tag="psb")
                    for ns in range(2):
                        nc.tensor.matmul(
                            ps_qk[:, ns * 512:(ns + 1) * 512],
                            wqk_sb[:, :, m * P:(m + 1) * P],
                            xn_sb[:, :, b, ns * 512:(ns + 1) * 512],
                            start=True, stop=True, perf_mode=DR)
                    nc.scalar.activation(out=qk_sb[:, m], in_=ps_qk,
                                         func=AF.Identity,
                                         bias=bqk_sb[:, m:m + 1])

            def emit_vt_mms(b, jps, ps_vts, vt_sb):
                for jp in jps:
                    ps_vt = psum_small.tile([P, 2, C], f32, tag="pss")
                    ps_vts[jp] = ps_vt
                    for j2 in range(2):
                        jt = 2 * jp + j2
                        nc.tensor.matmul(
                            ps_vt[:, j2],
                            xn_sb[:, :, b, jt * P:(jt + 1) * P],
                            wv_sb[:, :, :],
                            start=True, stop=True, perf_mode=DR)
                    # vt evacuates on DVE: keeps the ACT queue clear for exps
                    nc.vector.tensor_copy(
                        out=vt_sb[:, 2 * jp:2 * jp + 2], in_=ps_vt)

            def emit_s_part(b, es_sb, mts):
                src = qk_tiles[b]
                for mt in mts:
                    ps_s = psum_big.tile([P, N], f32, tag="psb")
                    for ns in range(2):
                        if has_bqk:
                            nc.tensor.matmul(
                                ps_s[:, ns * 512:(ns + 1) * 512],
                                src[:, 2:4, mt * P:(mt + 1) * P],
                                src[:, 0:2, ns * 512:(ns + 1) * 512],
                                start=True, stop=True, perf_mode=DR)
                        else:
                            nc.tensor.matmul(
                                ps_s[:, ns * 512:(ns + 1) * 512],
                                xn_sb[:, :, b, mt * P:(mt + 1) * P],
                                src[:, :, ns * 512:(ns + 1) * 512],
                                start=True, stop=True, perf_mode=DR)
                    nc.scalar.activation(out=es_sb[:, mt], in_=ps_s,
                                         func=AF.Exp, scale=ES_SCALE)

            def emit_sums_av(b, vt_sb, es_sb):
                # sums accumulate in two 1-bank pss halves so the big pool's
                # 12-slot rotation (mt0..7, u0, u1, av0, av1) stays stable:
                # u gets mid-stream-freed buffers (exp5/exp6) and only av
                # couples to the end of the exp stream.
                ps_sums = [psum_small.tile([P, 512], f32, tag="pss",
                                           name=f"ps_sum{_h}")
                           for _h in range(2)]
                xat_sb = xat_pool.tile([P, CT, N], fp8, tag="xat")
                for jp in range(4):
                    jsl = slice(2 * jp, 2 * jp + 2)
                    for ns in range(2):
                        hs = slice(ns * 512, (ns + 1) * 512)
                        nc.tensor.matmul(
                            ps_sums[ns], ones_sb, es_sb[:, jsl, hs],
                            start=(jp == 0), stop=(jp == 3), perf_mode=DR)
                ps_avs = [psum_big.tile([P, N], f32, tag="psb",
                                        name=f"ps_av{_i}") for _i in range(CT)]
                for ct2 in range(CT):
                    for jp in range(4):
                        jsl = slice(2 * jp, 2 * jp + 2)
                        for ns in range(2):
                            hs = slice(ns * 512, (ns + 1) * 512)
                            nc.tensor.matmul(
                                ps_avs[ct2][:, hs],
                                vt_sb[:, jsl, ct2 * P:(ct2 + 1) * P],
                                es_sb[:, jsl, hs],
                                start=(jp == 0), stop=(jp == 3), perf_mode=DR)
                r_sb = r_pool.tile([P, N], f32, tag="rr")
                for h in range(2):
                    hs = slice(h * 512, (h + 1) * 512)
                    nc.vector.reciprocal_approx_fast(out=r_sb[:, hs],
                                                     in_=ps_sums[h])
                # ct0 first, per half: frees av0's buffer earliest (the next
                # sample's S mt1 waits on it)
                for ct2 in range(CT):
                    for h in range(2):
                        hs = slice(h * 512, (h + 1) * 512)
                        if has_bv:
                            tmp = r_pool.tile([P, N], f32, tag="avtmp")
                            nc.vector.tensor_mul(tmp[:, hs],
                                                 ps_avs[ct2][:, hs],
                                                 r_sb[:, hs])
                            nc.vector.tensor_scalar_add(
                                xat_sb[:, ct2, hs], tmp[:, hs],
                                bv_sb[:, ct2:ct2 + 1])
                        else:
                            nc.vector.tensor_mul(xat_sb[:, ct2, hs],
                                                 ps_avs[ct2][:, hs],
                                                 r_sb[:, hs])
                return xat_sb

            def emit_proj_fuse(b, xat_sb):
                # proj runs in 1-bank pss quarters (keeps the big pool's
                # rotation to the S/u/av tiles); each quarter evacuates with
                # one fused DVE (y*gate + x) pass and DMAs immediately.
                out_ts = [out_pool.tile([P, N], f32, tag="outp",
                                        name=f"out_t{_i}") for _i in range(CT)]
                for h in range(2):
                    hs = slice(h * 512, (h + 1) * 512)
                    for ot in range(CT):
                        ps_yq = psum_small.tile([P, 512], f32, tag="pss",
                                                name=f"ps_y{ot}_{h}")
                        nc.tensor.matmul(
                            ps_yq,
                            wp_sb[:, :, ot * P:(ot + 1) * P],
                            xat_sb[:, :, hs],
                            start=True, stop=True, perf_mode=DR)
                        out_t = out_ts[ot]
                        nc.vector.affine_then_add(
                            out=out_t[:, hs], in0=ps_yq,
                            in1=x_sb[:, ot, b, hs],
                            scale=gate_sb[:, ot, b:b + 1], bias=0.0)
                        if has_bp:
                            nc.vector.tensor_scalar_add(
                                out_t[:, hs], out_t[:, hs],
                                bpg_sb[:, ot, b:b + 1])
                        nc.sync.dma_start(
                            out=out_d[b, ot * P:(ot + 1) * P, hs],
                            in_=out_t[:, hs])

            ES_SCALE = 0.0625 / (WS * WS) if has_bqk else 0.0625 / GS

            # ---- PE warm-up: dead matmuls during the DMA/stats head so
            # the HAM clock-gate reaches 8/8 before real matmuls ----
            def warm(nmm):
                ps_warm = psum_big.tile([P, 512], f32, tag="psb",
                                        name="ps_warm")
                for _ in range(nmm):
                    nc.tensor.matmul(ps_warm, warm_sb[:, 0:P], warm_sb,
                                     start=True, stop=True)

            def emit_first(b, both_act=False):
                if has_bqk:
                    emit_qk_biased(b)
                else:
                    emit_u(b, both_act=both_act)

            # ---- schedule: all stats/GN/SE work lives in the head or the
            # slack of earlier samples; the loop's steady state is paced by
            # the ACT exp stream. proj(b) is software-pipelined into
            # iteration b+1 (between S mt1 and mt2) so the next sample's S
            # stream never stalls behind proj's wait on the DVE evacuations.
            warm(12)
            emit_stats(0)
            emit_gn(0, xn_engines=("act", "dve"))
            emit_stats(1)
            emit_first(0, both_act=True)
            emit_gn(1, xn_engines=("pool", "pool"))
            nc.vector.tensor_scalar_mul(nb2_sb, b2_sb, -1.0)
            emit_se(0)
            emit_se(1)
            pend_proj = None
            for b in range(BL):
                vt_sb = vt_pool.tile([P, 8, C], fp8, tag="vt")
                ps_vts = {}
                emit_vt_mms(b, (0, 1), ps_vts, vt_sb)
                es_sb = es_pool.tile([P, 8, N], fp8, tag="es")
                emit_s_part(b, es_sb, (0, 1))
                if pend_proj is not None:
                    emit_proj_fuse(*pend_proj)
                    pend_proj = None
                emit_s_part(b, es_sb, (2, 3, 4, 5, 6, 7))
                emit_vt_mms(b, (2, 3), ps_vts, vt_sb)
                if b + 1 < BL:
                    emit_first(b + 1)
                xat_sb = emit_sums_av(b, vt_sb, es_sb)
                pend_proj = (b, xat_sb)
                if b < 2:
                    # prep sample b+2 while the pipeline runs: stats on DVE,
                    # xn on GpSimd -- all slack-tolerant (one-sample lead)
                    emit_stats(b + 2)
                    emit_gn(b + 2, xn_engines=("pool", "pool"))
                    emit_se(b + 2)
            emit_proj_fuse(*pend_proj)

    nc.compile()
    return nc


def _prep_inputs(x, gn_gamma, gn_beta, w_qkv, b_qkv, w_proj, b_proj,
                 w_se1, b_se1, w_se2, b_se2):
    fp8 = ml_dtypes.float8_e4m3
    f32 = np.float32

    def pt(w):  # [K, M] -> [128, K//128, M] partition-tiled
        K, M = w.shape
        return np.ascontiguousarray(w.reshape(K // P, P, M).transpose(1, 0, 2))

    wq = np.asarray(w_qkv[:C], np.float64)
    wk = np.asarray(w_qkv[C:2 * C], np.float64)
    # lhsT for u = G @ xn must be G^T[b, a] where G = Wk^T Wq, i.e. Wq^T Wk
    gt = (pt(wq.T @ wk) * GS).astype(fp8)
    wqk = (pt(np.ascontiguousarray(np.asarray(w_qkv[:512]).T)) * WS).astype(fp8)
    wv = (pt(np.ascontiguousarray(np.asarray(w_qkv[512:]).T)) * WS).astype(fp8)
    wp = (pt(np.ascontiguousarray(np.asarray(w_proj).T)) * WS).astype(fp8)
    w1 = pt(np.ascontiguousarray(np.asarray(w_se1).T)).astype(f32)
    w2 = np.ascontiguousarray(np.asarray(w_se2).T).astype(f32)

    def pcol(v):  # [256] -> [128, 2]
        return np.ascontiguousarray(np.asarray(v).reshape(2, P).T).astype(f32)

    gm = np.zeros((P, 16), f32)
    gm[np.arange(P), np.arange(P) // GSIZE] = 1.0
    shared = {
        "gt": gt, "wqk": wqk, "wv": wv, "wp": wp, "w1": w1, "w2": w2,
        "gamma": pcol(gn_gamma), "beta": pcol(gn_beta),
        # q/k biases land on 32x-scaled psums
        "bqk": np.ascontiguousarray(
            np.asarray(b_qkv[:512], f32).reshape(4, P).T) * WS,
        "bv": pcol(b_qkv[512:]) * WS,
        "bp": pcol(b_proj) * 1024.0,
        "b1": np.asarray(b_se1, f32).reshape(64, 1),
        "b2": pcol(b_se2),
        "gm": gm, "gmt": np.ascontiguousarray(gm.T),
    }
    xr = np.asarray(x, f32).reshape(B, C, N)
    in_maps = []
    for i in range(NCORES):
        m = dict(shared)
        m["x"] = np.ascontiguousarray(xr[i * BL:(i + 1) * BL])
        in_maps.append(m)
    flags = (bool(np.any(np.asarray(b_qkv[:512]) != 0)),
             bool(np.any(np.asarray(b_qkv[512:]) != 0)),
             bool(np.any(np.asarray(b_proj) != 0)))
    return in_maps, flags


def _get_program(flags):
    key = ("prog", flags)
    if key not in _CACHE:
        _CACHE[key] = _build_program(flags)
    return _CACHE[key]


def run(inputs, trace=False, trace_kwargs=None):
    """Build + run on all 8 cores. Returns (full_out, BassKernelResults)."""
    from concourse.bass_utils import run_bass_kernel_spmd

    in_maps, flags = _prep_inputs(**inputs)
    nc = _get_program(flags)
    kw = {}
    if trace:
        kw["trace"] = True
        if trace_kwargs:
            kw["trace_kwargs"] = trace_kwargs
    res = run_bass_kernel_spmd(nc, in_maps, list(range(NCORES)), **kw)
    out = np.concatenate([res.results[i]["out"] for i in range(NCORES)], axis=0)
    return out.reshape(B, C, HW, HW).astype(np.float32), res


def kernel(**inputs):
    out, _ = run(inputs, trace=False)
    return out


# revision 32
# speedup vs baseline: 1.0257x; 1.0257x over previous
"""AttentionBlock (GroupNorm + single-head spatial attention + SE gate + residual)
Trainium2 Bass/Tile kernel, data-parallel over batch across 8 NeuronCores.

Full shapes: x [32, 256, 32, 32] f32 -> out [32, 256, 32, 32] f32.
Per core: 4 samples. Per sample (C=256, N=1024), zero-bias fast path:
  xn = GroupNorm(x) (32 groups)            [C, N]  (fp8e4)
  u  = G @ xn, G = 256*Wk^T Wq (host fp8)  [C, N]  (fp8e4)  <- no separate q,k!
  esT = exp((xn^T u) / (16*256))           [N, N]  ([j, i] layout, fp8e4)
  vT = xn^T @ WvT                          [N, C]  (fp8e4)
  sums = ones @ esT  (accum over j)        [128, N]
  r = 1/sums (reciprocal_approx_fast)      [128, N]
  xat = (vT^T @ esT) * r                   [C, N]  (fp8e4)
  y = Wp @ xat                             [C, N]
  out = x + y * gate[c]                    (gate = SE sigmoid / 1024)

All attention matmuls are fp8e4 MatmulPerfMode.DoubleRow (two 128-deep
k-tiles per instruction, 2x bf16 rate: one 512-col matmul per ~216ns).
Host weights pre-scaled by 32 (Wv, Wp) / 256 (G) to sit in e4m3 range;
compensations fold into the exp scale and the SE gate (1/1024) for free.

The ACT engine is the pipeline limiter (8 [128,1024] exps + 2 u-evacs
per sample ~= 11.4us). Everything else is kept off ACT: vt/av/proj
evacuations on DVE, xn on GpSimd, and ALL GroupNorm/SE/stats work for
the 4 samples is hoisted into the DMA/warm-up head where DVE idles.
rstd = 1/sqrt(var+eps) is computed with a tiny Newton iteration on DVE
(GN var ~1) -- no Ln/Sqrt ACT table reloads mid-kernel.

If qkv biases are nonzero (not the case for this model's inputs) a
general program variant with explicit q,k evacuation is built instead.
"""

import numpy as np
import ml_dtypes

B, C, HW, N = 32, 256, 32, 1024
NCORES = 8
BL = B // NCORES          # samples per core
GROUPS = 32
GSIZE = C // GROUPS       # 8 channels per group
EPS = 1e-5
CT = 2                    # channel partition tiles (256 = 2*128)
P = 128
WS = 32.0                 # host-side fp8 weight scale (wv, wp)
GS = 256.0                # host-side fp8 scale for G = Wk^T Wq

_CACHE = {}


def _build_program(flags):
    has_bqk, has_bv, has_bp, has_gn = flags
    import concourse.bacc as bacc
    import concourse.mybir as mybir
    import concourse.tile as tile

    f32 = mybir.dt.float32
    fp8 = mybir.dt.float8e4
    bf16 = mybir.dt.bfloat16
    AX = mybir.AxisListType.X
    AF = mybir.ActivationFunctionType
    ALU = mybir.AluOpType
    DR = mybir.MatmulPerfMode.DoubleRow

    nc = bacc.Bacc()

    # ---- DRAM I/O ----
    x_d = nc.dram_tensor("x", [BL, C, N], f32, kind="ExternalInput")
    out_d = nc.dram_tensor("out", [BL, C, N], f32, kind="ExternalOutput")
    # gt = (Wq^T Wk)*GS partition-tiled (zero-bias path); wqk kept for the
    # general biased path
    gt_d = nc.dram_tensor("gt", [P, 2, C], fp8, kind="ExternalInput")
    wqk_d = nc.dram_tensor("wqk", [P, 2, 512], fp8, kind="ExternalInput")
    wv_d = nc.dram_tensor("wv", [P, 2, C], fp8, kind="ExternalInput")
    wp_d = nc.dram_tensor("wp", [P, 2, C], fp8, kind="ExternalInput")
    w1_d = nc.dram_tensor("w1", [P, 2, 64], f32, kind="ExternalInput")
    w2_d = nc.dram_tensor("w2", [64, C], f32, kind="ExternalInput")
    gamma_d = nc.dram_tensor("gamma", [P, 2], f32, kind="ExternalInput")
    beta_d = nc.dram_tensor("beta", [P, 2], f32, kind="ExternalInput")
    bqk_d = nc.dram_tensor("bqk", [P, 4], f32, kind="ExternalInput")
    bv_d = nc.dram_tensor("bv", [P, 2], f32, kind="ExternalInput")
    bp_d = nc.dram_tensor("bp", [P, 2], f32, kind="ExternalInput")
    b1_d = nc.dram_tensor("b1", [64, 1], f32, kind="ExternalInput")
    b2_d = nc.dram_tensor("b2", [P, 2], f32, kind="ExternalInput")
    gm_d = nc.dram_tensor("gm", [P, 32], f32, kind="ExternalInput")
    gmt_d = nc.dram_tensor("gmt", [16, P], f32, kind="ExternalInput")

    with tile.TileContext(nc) as tc:
        with (
            tc.tile_pool(name="persist", bufs=1) as persist,
            tc.tile_pool(name="uu", bufs=2) as u_pool,
            tc.tile_pool(name="vt", bufs=2) as vt_pool,
            tc.tile_pool(name="es", bufs=2) as es_pool,
            tc.tile_pool(name="xat", bufs=2) as xat_pool,
            tc.tile_pool(name="rr", bufs=2) as r_pool,
            tc.tile_pool(name="junk", bufs=2) as junk_pool,
            tc.tile_pool(name="outp", bufs=3) as out_pool,
            tc.tile_pool(name="psb", bufs=3, space="PSUM") as psum_big,
            tc.tile_pool(name="pss", bufs=2, space="PSUM") as psum_small,
        ):
            # ---- DMA prologue: x slices first (sample 0's stats are the
            # critical path), then consts/weights in first-use order.
            # warm-up constants first: the DVE memset must head the DVE
            # queue (no DMA dependency) so the PE warm matmuls start ~1.5us
            warm_sb = persist.tile([P, 512], bf16)
            nc.vector.memset(warm_sb, 1.0)
            ones_sb = persist.tile([P, 2, P], fp8)
            nc.gpsimd.memset(ones_sb, 1.0)

            x_sb = persist.tile([P, CT, BL, N], f32)

            def load_x(b):
                for ct in range(CT):
                    nc.sync.dma_start(out=x_sb[:, ct, b],
                                      in_=x_d[b, ct * P:(ct + 1) * P, :])

            load_x(0)
            gm_sb = persist.tile([P, 32], f32)
            nc.sync.dma_start(out=gm_sb, in_=gm_d[:, :])
            gmt_sb = persist.tile([16, P], f32)
            nc.sync.dma_start(out=gmt_sb, in_=gmt_d[:, :])
            load_x(1)
            gamma_sb = persist.tile([P, 2], f32)
            nc.sync.dma_start(out=gamma_sb, in_=gamma_d[:, :])
            beta_sb = persist.tile([P, 2], f32)
            nc.sync.dma_start(out=beta_sb, in_=beta_d[:, :])
            if has_bqk:
                wqk_sb = persist.tile([P, 2, 512], fp8)
                nc.sync.dma_start(out=wqk_sb, in_=wqk_d[:, :, :])
                bqk_sb = persist.tile([P, 4], f32)
                nc.sync.dma_start(out=bqk_sb, in_=bqk_d[:, :])
            else:
                gt_sb = persist.tile([P, 2, C], fp8)
                nc.sync.dma_start(out=gt_sb, in_=gt_d[:, :, :])
            wv_sb = persist.tile([P, 2, C], fp8)
            nc.sync.dma_start(out=wv_sb, in_=wv_d[:, :, :])
            load_x(2)
            load_x(3)
            bv_sb = persist.tile([P, 2], f32)
            nc.sync.dma_start(out=bv_sb, in_=bv_d[:, :])
            bp_sb = persist.tile([P, 2], f32)
            nc.sync.dma_start(out=bp_sb, in_=bp_d[:, :])
            b1_sb = persist.tile([64, 1], f32)
            nc.sync.dma_start(out=b1_sb, in_=b1_d[:, :])
            b2_sb = persist.tile([P, 2], f32)
            nc.sync.dma_start(out=b2_sb, in_=b2_d[:, :])
            w1_sb = persist.tile([P, 2, 64], f32)
            nc.sync.dma_start(out=w1_sb, in_=w1_d[:, :, :])
            w2_sb = persist.tile([64, C], f32)
            nc.sync.dma_start(out=w2_sb, in_=w2_d[:, :])
            wp_sb = persist.tile([P, 2, C], fp8)
            nc.sync.dma_start(out=wp_sb, in_=wp_d[:, :, :])

            nb2_sb = persist.tile([P, 2], f32)

            # ---- persistent intermediates ----
            bn_sb = persist.tile([P, CT, BL, 12], f32)  # bn_stats (2 seg x 6)
            msum_c = persist.tile([P, CT, BL], f32)     # channel sum / 256
            s2_c = persist.tile([P, CT, BL], f32)       # channel sum of x^2
            cv_c = persist.tile([P, CT, BL], f32)       # sum of count*var
            ab_sb = persist.tile([P, BL, 4], f32)       # [a-ct0,a-ct1,b-ct0,b-ct1]
            xn_sb = persist.tile([P, CT, BL, N], fp8)
            gate_sb = persist.tile([P, CT, BL], f32)    # sigmoid/1024
            bpg_sb = persist.tile([P, CT, BL], f32)     # bp*sigmoid (bias case)
            h1_sb = persist.tile([64, BL], f32)
            qk_tiles = [None] * BL

            def emit_stats_bn(b):
                for ct in range(CT):
                    for seg in range(2):
                        nc.vector.bn_stats(
                            out=bn_sb[:, ct, b, seg * 6:(seg + 1) * 6],
                            in_=x_sb[:, ct, b, seg * 512:(seg + 1) * 512])

            def emit_stats(b):
                # bn_stats per 512-elem segment gives count/mean/count*var
                # for even/odd interleaves in one DVE read of x.
                # sum_c = 256*sum(means); sumsq_c = sum(cv) + 256*sum(mean^2)
                for ct in range(CT):
                    means = bn_sb[:, ct, b, 1::3]   # [P, 4] stride 3
                    cvs = bn_sb[:, ct, b, 2::3]     # [P, 4]
                    nc.vector.reduce_sum(
                        out=msum_c[:, ct, b:b + 1], in_=means, axis=AX)
                    nc.vector.reduce_sum(
                        out=cv_c[:, ct, b:b + 1], in_=cvs, axis=AX)
                    jt = junk_pool.tile([P, 4], f32, tag="junk4")
                    msq = junk_pool.tile([P, 1], f32, tag="junk1")
                    nc.vector.affine_mul_reduce(
                        out=jt, accum_out=msq, in0=means, in1=means,
                        scale=1.0, bias=0.0)
                    nc.vector.affine_then_add(
                        out=s2_c[:, ct, b:b + 1], in0=msq,
                        in1=cv_c[:, ct, b:b + 1], scale=256.0, bias=0.0)

            def emit_gn(b, xn_engines):
                # per-sample GroupNorm coefficients (a, bb) + xn write.
                # gm columns are host-prescaled: ps_g = [-mean | E[x^2]]
                ps_g = psum_small.tile([16, 4], f32, tag="pss")
                for ct in range(CT):
                    nc.tensor.matmul(ps_g[:, ct:ct + 1], gm_sb[:, 0:16],
                                     msum_c[:, ct, b:b + 1],
                                     start=True, stop=True)
                    nc.tensor.matmul(ps_g[:, 2 + ct:3 + ct], gm_sb[:, 16:32],
                                     s2_c[:, ct, b:b + 1],
                                     start=True, stop=True)
                nmean = persist.tile([16, 2], f32)
                nc.vector.tensor_copy(out=nmean, in_=ps_g[:, 0:2])
                msq = persist.tile([16, 2], f32)
                nc.vector.tensor_mul(msq, nmean, nmean)
                vpe = persist.tile([16, 2], f32)
                nc.vector.tensor_sub(vpe, ps_g[:, 2:4], msq)
                nc.vector.tensor_scalar_add(vpe, vpe, EPS)
                # rstd = 1/sqrt(var+eps): GN var ~1 so z0 = 1.5-(var+eps)/2
                # is within 0.4%; one Newton step reaches ~2e-5.
                rsm = persist.tile([16, 4], f32)
                z = rsm[:, 0:2]
                nc.vector.tensor_scalar(out=z, in0=vpe, scalar1=-0.5,
                                        scalar2=1.5, op0=ALU.mult, op1=ALU.add)
                zt = persist.tile([16, 2], f32)
                nc.vector.tensor_mul(zt, z, z)
                nc.vector.tensor_mul(zt, zt, vpe)
                nc.vector.tensor_scalar(out=zt, in0=zt, scalar1=-0.5,
                                        scalar2=1.5, op0=ALU.mult,
                                        op1=ALU.add)
                nc.vector.tensor_mul(z, z, zt)
                nc.vector.tensor_mul(rsm[:, 2:4], nmean, z)
                ps_bc = psum_small.tile([P, 4], f32, tag="pss")
                nc.tensor.matmul(ps_bc, gmt_sb, rsm, start=True, stop=True)
                if has_gn:
                    for ct in range(CT):
                        nc.vector.tensor_scalar_mul(
                            ab_sb[:, b, ct:ct + 1], ps_bc[:, ct:ct + 1],
                            gamma_sb[:, ct:ct + 1])
                        nc.vector.tensor_scalar(
                            out=ab_sb[:, b, 2 + ct:3 + ct],
                            in0=ps_bc[:, 2 + ct:3 + ct],
                            scalar1=gamma_sb[:, ct:ct + 1],
                            scalar2=beta_sb[:, ct:ct + 1],
                            op0=ALU.mult, op1=ALU.add)
                else:
                    nc.vector.tensor_copy(out=ab_sb[:, b], in_=ps_bc)
                for ct in range(CT):
                    eng = xn_engines[ct]
                    a_col = ab_sb[:, b, ct:ct + 1]
                    b_col = ab_sb[:, b, 2 + ct:3 + ct]
                    if eng == "act":
                        nc.scalar.activation(
                            out=xn_sb[:, ct, b], in_=x_sb[:, ct, b],
                            func=AF.Identity, bias=b_col, scale=a_col)
                    elif eng == "dve":
                        nc.vector.tensor_scalar(
                            out=xn_sb[:, ct, b], in0=x_sb[:, ct, b],
                            scalar1=a_col, scalar2=b_col,
                            op0=ALU.mult, op1=ALU.add)
                    else:
                        nc.gpsimd.tensor_scalar(
                            out=xn_sb[:, ct, b], in0=x_sb[:, ct, b],
                            scalar1=a_col, scalar2=b_col,
                            op0=ALU.mult, op1=ALU.add)

            def emit_se(b):
                # sigmoid(z)/1024 = 1/(1024*(1+exp(-z))): stays in exp table
                # and folds the fp8 weight-scale compensation in for free.
                ps_h1 = psum_small.tile([64, 1], f32, tag="pss")
                for ct in range(CT):
                    nc.tensor.matmul(ps_h1, w1_sb[:, ct],
                                     msum_c[:, ct, b:b + 1],
                                     start=(ct == 0), stop=(ct == 1))
                # pooled = msum*256/1024 -> relu scale 0.25
                nc.scalar.activation(out=h1_sb[:, b:b + 1], in_=ps_h1,
                                     func=AF.Relu, bias=b1_sb[:, 0:1],
                                     scale=0.25)
                for ot in range(CT):
                    ps_gate = psum_small.tile([P, 1], f32, tag="pss")
                    nc.tensor.matmul(ps_gate, w2_sb[:, ot * P:(ot + 1) * P],
                                     h1_sb[:, b:b + 1], start=True, stop=True)
                    eg = persist.tile([P, 1], f32)
                    nc.scalar.activation(out=eg, in_=ps_gate, func=AF.Exp,
                                         scale=-1.0, bias=nb2_sb[:, ot:ot + 1])
                    nc.vector.tensor_scalar(
                        out=eg, in0=eg, scalar1=1024.0, scalar2=1024.0,
                        op0=ALU.mult, op1=ALU.add)
                    nc.vector.reciprocal(gate_sb[:, ot, b:b + 1], eg)
                    if has_bp:
                        # bp_sb holds 1024*bp -> bpg = bp*sigmoid
                        nc.vector.tensor_scalar_mul(bpg_sb[:, ot, b:b + 1],
                                                    gate_sb[:, ot, b:b + 1],
                                                    bp_sb[:, ot:ot + 1])

            def emit_u(b, both_act=False):
                # zero-bias path: u = (G/GS) @ xn so that S = xn^T u.
                # Two fp8 tiles replace four (q0,q1,k0,k1) evacuations;
                # copies split ACT/DVE to balance the two evac engines.
                u_sb = u_pool.tile([P, 2, N], fp8, tag="uu")
                qk_tiles[b] = u_sb
                for at in range(CT):
                    ps_u = psum_big.tile([P, N], f32, tag="psb")
                    for ns in range(2):
                        nc.tensor.matmul(
                            ps_u[:, ns * 512:(ns + 1) * 512],
                            gt_sb[:, :, at * P:(at + 1) * P],
                            xn_sb[:, :, b, ns * 512:(ns + 1) * 512],
                            start=True, stop=True, perf_mode=DR)
                    if at == 0 or both_act:
                        nc.scalar.copy(out=u_sb[:, at], in_=ps_u)
                    else:
                        nc.vector.tensor_copy(out=u_sb[:, at], in_=ps_u)

            def emit_qk_biased(b):
                # general path with nonzero qkv bias: q, k : [c, n] fp8
                qk_sb = u_pool.tile([P, 4, N], fp8, tag="uu")
                qk_tiles[b] = qk_sb
                for m in (2, 3, 0, 1):
                    ps_qk = psum_big.tile([P, N], f32, tag="psb")
                    for ns in range(2):
                        nc.tensor.matmul(
                            ps_qk[:, ns * 512:(ns + 1) * 512],
                            wqk_sb[:, :, m * P:(m + 1) * P],
                            xn_sb[:, :, b, ns * 512:(ns + 1) * 512],
                            start=True, stop=True, perf_mode=DR)
                    nc.scalar.activation(out=qk_sb[:, m], in_=ps_qk,
                                         func=AF.Identity,
                                         bias=bqk_sb[:, m:m + 1])

            def emit_vt_mms(b, jps, ps_vts, vt_sb):
                for jp in jps:
                    ps_vt = psum_small.tile([P, 2, C], f32, tag="pss")
                    ps_vts[jp] = ps_vt
                    for j2 in range(2):
                        jt = 2 * jp + j2
                        nc.tensor.matmul(
                            ps_vt[:, j2],
                            xn_sb[:, :, b, jt * P:(jt + 1) * P],
                            wv_sb[:, :, :],
                            start=True, stop=True, perf_mode=DR)
                    # vt evacuates on DVE: keeps the ACT queue clear for exps
                    nc.vector.tensor_copy(
                        out=vt_sb[:, 2 * jp:2 * jp + 2], in_=ps_vt)

            def emit_s_part(b, es_sb, mts):
                src = qk_tiles[b]
                for mt in mts:
                    ps_s = psum_big.tile([P, N], f32, tag="psb")
                    for ns in range(2):
                        if has_bqk:
                            nc.tensor.matmul(
                                ps_s[:, ns * 512:(ns + 1) * 512],
                                src[:, 2:4, mt * P:(mt + 1) * P],
                                src[:, 0:2, ns * 512:(ns + 1) * 512],
                                start=True, stop=True, perf_mode=DR)
                        else:
                            nc.tensor.matmul(
                                ps_s[:, ns * 512:(ns + 1) * 512],
                                xn_sb[:, :, b, mt * P:(mt + 1) * P],
                                src[:, :, ns * 512:(ns + 1) * 512],
                                start=True, stop=True, perf_mode=DR)
                    nc.scalar.activation(out=es_sb[:, mt], in_=ps_s,
                                         func=AF.Exp, scale=ES_SCALE)

            def emit_sums_av(b, vt_sb, es_sb):
                # sums accumulate in two 1-bank pss halves so the big pool's
                # 12-slot rotation (mt0..7, u0, u1, av0, av1) stays stable:
                # u gets mid-stream-freed buffers (exp5/exp6) and only av
                # couples to the end of the exp stream.
                ps_sums = [psum_small.tile([P, 512], f32, tag="pss",
                                           name=f"ps_sum{_h}")
                           for _h in range(2)]
                xat_sb = xat_pool.tile([P, CT, N], fp8, tag="xat")
                for jp in range(4):
                    jsl = slice(2 * jp, 2 * jp + 2)
                    for ns in range(2):
                        hs = slice(ns * 512, (ns + 1) * 512)
                        nc.tensor.matmul(
                            ps_sums[ns], ones_sb, es_sb[:, jsl, hs],
                            start=(jp == 0), stop=(jp == 3), perf_mode=DR)
                ps_avs = [psum_big.tile([P, N], f32, tag="psb",
                                        name=f"ps_av{_i}") for _i in range(CT)]
                for ct2 in range(CT):
                    for jp in range(4):
                        jsl = slice(2 * jp, 2 * jp + 2)
                        for ns in range(2):
                            hs = slice(ns * 512, (ns + 1) * 512)
                            nc.tensor.matmul(
                                ps_avs[ct2][:, hs],
                                vt_sb[:, jsl, ct2 * P:(ct2 + 1) * P],
                                es_sb[:, jsl, hs],
                                start=(jp == 0), stop=(jp == 3), perf_mode=DR)
                r_sb = r_pool.tile([P, N], f32, tag="rr")
                for h in range(2):
                    hs = slice(h * 512, (h + 1) * 512)
                    nc.vector.reciprocal_approx_fast(out=r_sb[:, hs],
                                                     in_=ps_sums[h])
                # ct0 first, per half: frees av0's buffer earliest (the next
                # sample's S mt1 waits on it)
                for ct2 in range(CT):
                    for h in range(2):
                        hs = slice(h * 512, (h + 1) * 512)
                        if has_bv:
                            tmp = r_pool.tile([P, N], f32, tag="avtmp")
                            nc.vector.tensor_mul(tmp[:, hs],
                                                 ps_avs[ct2][:, hs],
                                                 r_sb[:, hs])
                            nc.vector.tensor_scalar_add(
                                xat_sb[:, ct2, hs], tmp[:, hs],
                                bv_sb[:, ct2:ct2 + 1])
                        else:
                            nc.vector.tensor_mul(xat_sb[:, ct2, hs],
                                                 ps_avs[ct2][:, hs],
                                                 r_sb[:, hs])
                return xat_sb

            def emit_proj_fuse(b, xat_sb):
                # proj runs in 1-bank pss quarters (keeps the big pool's
                # rotation to the S/u/av tiles); each quarter evacuates with
                # one fused DVE (y*gate + x) pass and DMAs immediately.
                out_ts = [out_pool.tile([P, N], f32, tag="outp",
                                        name=f"out_t{_i}") for _i in range(CT)]
                for h in range(2):
                    hs = slice(h * 512, (h + 1) * 512)
                    for ot in range(CT):
                        ps_yq = psum_small.tile([P, 512], f32, tag="pss",
                                                name=f"ps_y{ot}_{h}")
                        nc.tensor.matmul(
                            ps_yq,
                            wp_sb[:, :, ot * P:(ot + 1) * P],
                            xat_sb[:, :, hs],
                            start=True, stop=True, perf_mode=DR)
                        out_t = out_ts[ot]
                        nc.vector.affine_then_add(
                            out=out_t[:, hs], in0=ps_yq,
                            in1=x_sb[:, ot, b, hs],
                            scale=gate_sb[:, ot, b:b + 1], bias=0.0)
                        if has_bp:
                            nc.vector.tensor_scalar_add(
                                out_t[:, hs], out_t[:, hs],
                                bpg_sb[:, ot, b:b + 1])
                        nc.sync.dma_start(
                            out=out_d[b, ot * P:(ot + 1) * P, hs],
                            in_=out_t[:, hs])

            ES_SCALE = 0.0625 / (WS * WS) if has_bqk else 0.0625 / GS

            # ---- PE warm-up: dead matmuls during the DMA/stats head so
            # the HAM clock-gate reaches 8/8 before real matmuls ----
            def warm(nmm):
                ps_warm = psum_big.tile([P, 512], f32, tag="psb",
                                        name="ps_warm")
                for _ in range(nmm):
                    nc.tensor.matmul(ps_warm, warm_sb[:, 0:P], warm_sb,
                                     start=True, stop=True)

            def emit_first(b, both_act=False):
                if has_bqk:
                    emit_qk_biased(b)
                else:
                    emit_u(b, both_act=both_act)

            # ---- schedule: all stats/GN/SE work lives in the head or the
            # slack of earlier samples; the loop's steady state is paced by
            # the ACT exp stream. proj(b) is software-pipelined into
            # iteration b+1 (between S mt1 and mt2) so the next sample's S
            # stream never stalls behind proj's wait on the DVE evacuations.
            warm(12)
            emit_stats_bn(0)
            emit_stats(0)
            emit_gn(0, xn_engines=("act", "dve"))
            emit_stats_bn(1)
            emit_stats(1)
            emit_first(0, both_act=True)
            emit_gn(1, xn_engines=("pool", "pool"))
            nc.vector.tensor_scalar_mul(nb2_sb, b2_sb, -1.0)
            emit_se(0)
            emit_se(1)
            pend_proj = None
            for b in range(BL):
                vt_sb = vt_pool.tile([P, 8, C], fp8, tag="vt")
                ps_vts = {}
                emit_vt_mms(b, (0, 1), ps_vts, vt_sb)
                es_sb = es_pool.tile([P, 8, N], fp8, tag="es")
                emit_s_part(b, es_sb, (0, 1))
                if pend_proj is not None:
                    emit_proj_fuse(*pend_proj)
                    pend_proj = None
                emit_s_part(b, es_sb, (2, 3))
                if b < 2:
                    # the bn passes for sample b+2 run mid-exp-stream so the
                    # later gn matmuls find their DVE inputs ready at the
                    # sample boundary instead of stalling the next S stream
                    emit_stats_bn(b + 2)
                emit_s_part(b, es_sb, (4, 5, 6, 7))
                emit_vt_mms(b, (2, 3), ps_vts, vt_sb)
                if b + 1 < BL:
                    emit_first(b + 1)
                else:
                    # keep the big pool's 12-slot rotation period on the last
                    # iteration (no u tiles): placeholder allocations only
                    psum_big.tile([P, N], f32, tag="psb", name="pad0")
                    psum_big.tile([P, N], f32, tag="psb", name="pad1")
                xat_sb = emit_sums_av(b, vt_sb, es_sb)
                pend_proj = (b, xat_sb)
                if b < 2:
                    emit_stats(b + 2)
                    emit_gn(b + 2, xn_engines=("pool", "pool"))
                    emit_se(b + 2)
            emit_proj_fuse(*pend_proj)

    nc.compile()
    return nc


def _prep_inputs(x, gn_gamma, gn_beta, w_qkv, b_qkv, w_proj, b_proj,
                 w_se1, b_se1, w_se2, b_se2):
    fp8 = ml_dtypes.float8_e4m3
    f32 = np.float32

    def pt(w):  # [K, M] -> [128, K//128, M] partition-tiled
        K, M = w.shape
        return np.ascontiguousarray(w.reshape(K // P, P, M).transpose(1, 0, 2))

    wq = np.asarray(w_qkv[:C], np.float64)
    wk = np.asarray(w_qkv[C:2 * C], np.float64)
    # lhsT for u = G @ xn must be G^T[b, a] where G = Wk^T Wq, i.e. Wq^T Wk
    gt = (pt(wq.T @ wk) * GS).astype(fp8)
    wqk = (pt(np.ascontiguousarray(np.asarray(w_qkv[:512]).T)) * WS).astype(fp8)
    wv = (pt(np.ascontiguousarray(np.asarray(w_qkv[512:]).T)) * WS).astype(fp8)
    wp = (pt(np.ascontiguousarray(np.asarray(w_proj).T)) * WS).astype(fp8)
    w1 = pt(np.ascontiguousarray(np.asarray(w_se1).T)).astype(f32)
    w2 = np.ascontiguousarray(np.asarray(w_se2).T).astype(f32)

    def pcol(v):  # [256] -> [128, 2]
        return np.ascontiguousarray(np.asarray(v).reshape(2, P).T).astype(f32)

    gm1 = np.zeros((P, 16), f32)
    gm1[np.arange(P), np.arange(P) // GSIZE] = 1.0
    gm = np.concatenate([gm1 * (-256.0 / (GSIZE * N)),
                         gm1 * (1.0 / (GSIZE * N))], axis=1).astype(f32)
    shared = {
        "gt": gt, "wqk": wqk, "wv": wv, "wp": wp, "w1": w1, "w2": w2,
        "gamma": pcol(gn_gamma), "beta": pcol(gn_beta),
        # q/k biases land on 32x-scaled psums
        "bqk": np.ascontiguousarray(
            np.asarray(b_qkv[:512], f32).reshape(4, P).T) * WS,
        "bv": pcol(b_qkv[512:]) * WS,
        "bp": pcol(b_proj) * 1024.0,
        "b1": np.asarray(b_se1, f32).reshape(64, 1),
        "b2": pcol(b_se2),
        "gm": gm, "gmt": np.ascontiguousarray(gm1.T),
    }
    xr = np.asarray(x, f32).reshape(B, C, N)
    in_maps = []
    for i in range(NCORES):
        m = dict(shared)
        m["x"] = np.ascontiguousarray(xr[i * BL:(i + 1) * BL])
        in_maps.append(m)
    flags = (bool(np.any(np.asarray(b_qkv[:512]) != 0)),
             bool(np.any(np.asarray(b_qkv[512:]) != 0)),
             bool(np.any(np.asarray(b_proj) != 0)),
             bool(np.any(np.asarray(gn_gamma) != 1) or
                  np.any(np.asarray(gn_beta) != 0)))
    return in_maps, flags


def _get_program(flags):
    key = ("prog", flags)
    if key not in _CACHE:
        _CACHE[key] = _build_program(flags)
    return _CACHE[key]


def run(inputs, trace=False, trace_kwargs=None):
    """Build + run on all 8 cores. Returns (full_out, BassKernelResults)."""
    from concourse.bass_utils import run_bass_kernel_spmd

    in_maps, flags = _prep_inputs(**inputs)
    nc = _get_program(flags)
    kw = {}
    if trace:
        kw["trace"] = True
        if trace_kwargs:
            kw["trace_kwargs"] = trace_kwargs
    res = run_bass_kernel_spmd(nc, in_maps, list(range(NCORES)), **kw)
    out = np.concatenate([res.results[i]["out"] for i in range(NCORES)], axis=0)
    return out.reshape(B, C, HW, HW).astype(np.float32), res


def kernel(**inputs):
    out, _ = run(inputs, trace=False)
    return out


# revision 33
# speedup vs baseline: 1.0846x; 1.0574x over previous
"""AttentionBlock (GroupNorm + single-head spatial attention + SE gate + residual)
Trainium2 Bass/Tile kernel, data-parallel over batch across 8 NeuronCores.

Full shapes: x [32, 256, 32, 32] f32 -> out [32, 256, 32, 32] f32.
Per core: 4 samples. Per sample (C=256, N=1024), zero-bias fast path:
  xn = GroupNorm(x) (32 groups)            [C, N]  (fp8e4)
  u  = G @ xn, G = 256*Wk^T Wq (host fp8)  [C, N]  (fp8e4)  <- no separate q,k!
  esT = exp((xn^T u) / (16*256))           [N, N]  ([j, i] layout, fp8e4)
  vT = xn^T @ WvT                          [N, C]  (fp8e4)
  sums = ones @ esT  (accum over j)        [128, N]
  r = 1/sums (reciprocal_approx_fast)      [128, N]
  xat = (vT^T @ esT) * r                   [C, N]  (fp8e4)
  y = Wp @ xat                             [C, N]
  out = x + y * gate[c]                    (gate = SE sigmoid / 1024)

All attention matmuls are fp8e4 MatmulPerfMode.DoubleRow (two 128-deep
k-tiles per instruction, 2x bf16 rate: one 512-col matmul per ~216ns).
Host weights pre-scaled by 32 (Wv, Wp) / 256 (G) to sit in e4m3 range;
compensations fold into the exp scale and the SE gate (1/1024) for free.

The ACT engine is the pipeline limiter (8 [128,1024] exps + 2 u-evacs
per sample ~= 11.4us). Everything else is kept off ACT: vt/av/proj
evacuations on DVE, xn on GpSimd, and ALL GroupNorm/SE/stats work for
the 4 samples is hoisted into the DMA/warm-up head where DVE idles.
rstd = 1/sqrt(var+eps) is computed with a tiny Newton iteration on DVE
(GN var ~1) -- no Ln/Sqrt ACT table reloads mid-kernel.

If qkv biases are nonzero (not the case for this model's inputs) a
general program variant with explicit q,k evacuation is built instead.
"""

import numpy as np
import ml_dtypes

B, C, HW, N = 32, 256, 32, 1024
NCORES = 8
BL = B // NCORES          # samples per core
GROUPS = 32
GSIZE = C // GROUPS       # 8 channels per group
EPS = 1e-5
CT = 2                    # channel partition tiles (256 = 2*128)
P = 128
WS = 32.0                 # host-side fp8 weight scale (wv, wp)
GS = 256.0                # host-side fp8 scale for G = Wk^T Wq

_CACHE = {}


def _build_program(flags):
    has_bqk, has_bv, has_bp, has_gn = flags
    import concourse.bacc as bacc
    import concourse.mybir as mybir
    import concourse.tile as tile

    f32 = mybir.dt.float32
    fp8 = mybir.dt.float8e4
    bf16 = mybir.dt.bfloat16
    AX = mybir.AxisListType.X
    AF = mybir.ActivationFunctionType
    ALU = mybir.AluOpType
    DR = mybir.MatmulPerfMode.DoubleRow

    nc = bacc.Bacc()

    # ---- DRAM I/O ----
    x_d = nc.dram_tensor("x", [BL, C, N], f32, kind="ExternalInput")
    out_d = nc.dram_tensor("out", [BL, C, N], f32, kind="ExternalOutput")
    # gt = (Wq^T Wk)*GS partition-tiled (zero-bias path); wqk kept for the
    # general biased path
    gt_d = nc.dram_tensor("gt", [P, 2, C], fp8, kind="ExternalInput")
    wqk_d = nc.dram_tensor("wqk", [P, 2, 512], fp8, kind="ExternalInput")
    wv_d = nc.dram_tensor("wv", [P, 2, C], fp8, kind="ExternalInput")
    wp_d = nc.dram_tensor("wp", [P, 2, C], fp8, kind="ExternalInput")
    w1_d = nc.dram_tensor("w1", [P, 2, 64], f32, kind="ExternalInput")
    w2_d = nc.dram_tensor("w2", [64, C], f32, kind="ExternalInput")
    gamma_d = nc.dram_tensor("gamma", [P, 2], f32, kind="ExternalInput")
    beta_d = nc.dram_tensor("beta", [P, 2], f32, kind="ExternalInput")
    bqk_d = nc.dram_tensor("bqk", [P, 4], f32, kind="ExternalInput")
    bv_d = nc.dram_tensor("bv", [P, 2], f32, kind="ExternalInput")
    bp_d = nc.dram_tensor("bp", [P, 2], f32, kind="ExternalInput")
    b1_d = nc.dram_tensor("b1", [64, 1], f32, kind="ExternalInput")
    b2_d = nc.dram_tensor("b2", [P, 2], f32, kind="ExternalInput")
    gm_d = nc.dram_tensor("gm", [P, 32], f32, kind="ExternalInput")
    gmt_d = nc.dram_tensor("gmt", [16, P], f32, kind="ExternalInput")

    with tile.TileContext(nc) as tc:
        with (
            tc.tile_pool(name="persist", bufs=1) as persist,
            tc.tile_pool(name="uu", bufs=2) as u_pool,
            tc.tile_pool(name="vt", bufs=2) as vt_pool,
            tc.tile_pool(name="es", bufs=2) as es_pool,
            tc.tile_pool(name="xat", bufs=2) as xat_pool,
            tc.tile_pool(name="rr", bufs=2) as r_pool,
            tc.tile_pool(name="junk", bufs=2) as junk_pool,
            tc.tile_pool(name="outp", bufs=3) as out_pool,
            tc.tile_pool(name="psb", bufs=3, space="PSUM") as psum_big,
            tc.tile_pool(name="pss", bufs=2, space="PSUM") as psum_small,
        ):
            # ---- DMA prologue: x slices first (sample 0's stats are the
            # critical path), then consts/weights in first-use order.
            # warm-up constants first: the DVE memset must head the DVE
            # queue (no DMA dependency) so the PE warm matmuls start ~1.5us
            warm_sb = persist.tile([P, 512], bf16)
            nc.vector.memset(warm_sb, 1.0)
            ones_sb = persist.tile([P, 2, P], fp8)
            nc.gpsimd.memset(ones_sb, 1.0)

            x_sb = persist.tile([P, CT, BL, N], f32)

            def load_x(b):
                for ct in range(CT):
                    nc.sync.dma_start(out=x_sb[:, ct, b],
                                      in_=x_d[b, ct * P:(ct + 1) * P, :])

            load_x(0)
            gm_sb = persist.tile([P, 32], f32)
            nc.sync.dma_start(out=gm_sb, in_=gm_d[:, :])
            gmt_sb = persist.tile([16, P], f32)
            nc.sync.dma_start(out=gmt_sb, in_=gmt_d[:, :])
            load_x(1)
            gamma_sb = persist.tile([P, 2], f32)
            nc.sync.dma_start(out=gamma_sb, in_=gamma_d[:, :])
            beta_sb = persist.tile([P, 2], f32)
            nc.sync.dma_start(out=beta_sb, in_=beta_d[:, :])
            if has_bqk:
                wqk_sb = persist.tile([P, 2, 512], fp8)
                nc.sync.dma_start(out=wqk_sb, in_=wqk_d[:, :, :])
                bqk_sb = persist.tile([P, 4], f32)
                nc.sync.dma_start(out=bqk_sb, in_=bqk_d[:, :])
            else:
                gt_sb = persist.tile([P, 2, C], fp8)
                nc.sync.dma_start(out=gt_sb, in_=gt_d[:, :, :])
            wv_sb = persist.tile([P, 2, C], fp8)
            nc.sync.dma_start(out=wv_sb, in_=wv_d[:, :, :])
            load_x(2)
            load_x(3)
            bv_sb = persist.tile([P, 2], f32)
            nc.sync.dma_start(out=bv_sb, in_=bv_d[:, :])
            bp_sb = persist.tile([P, 2], f32)
            nc.sync.dma_start(out=bp_sb, in_=bp_d[:, :])
            b1_sb = persist.tile([64, 1], f32)
            nc.sync.dma_start(out=b1_sb, in_=b1_d[:, :])
            b2_sb = persist.tile([P, 2], f32)
            nc.sync.dma_start(out=b2_sb, in_=b2_d[:, :])
            w1_sb = persist.tile([P, 2, 64], f32)
            nc.sync.dma_start(out=w1_sb, in_=w1_d[:, :, :])
            w2_sb = persist.tile([64, C], f32)
            nc.sync.dma_start(out=w2_sb, in_=w2_d[:, :])
            wp_sb = persist.tile([P, 2, C], fp8)
            nc.sync.dma_start(out=wp_sb, in_=wp_d[:, :, :])

            nb2_sb = persist.tile([P, 2], f32)

            # ---- persistent intermediates ----
            bn_sb = persist.tile([P, CT, BL, 12], f32)  # bn_stats (2 seg x 6)
            msum_c = persist.tile([P, CT, BL], f32)     # channel sum / 256
            s2_c = persist.tile([P, CT, BL], f32)       # channel sum of x^2
            cv_c = persist.tile([P, CT, BL], f32)       # sum of count*var
            ab_sb = persist.tile([P, BL, 4], f32)       # [a-ct0,a-ct1,b-ct0,b-ct1]
            xn_sb = persist.tile([P, CT, BL, N], fp8)
            gate_sb = persist.tile([P, CT, BL], f32)    # sigmoid/1024
            bpg_sb = persist.tile([P, CT, BL], f32)     # bp*sigmoid (bias case)
            h1_sb = persist.tile([64, BL], f32)
            qk_tiles = [None] * BL

            def emit_stats_bn(b):
                for ct in range(CT):
                    for seg in range(2):
                        nc.vector.bn_stats(
                            out=bn_sb[:, ct, b, seg * 6:(seg + 1) * 6],
                            in_=x_sb[:, ct, b, seg * 512:(seg + 1) * 512])

            def emit_stats(b):
                # bn_stats per 512-elem segment gives count/mean/count*var
                # for even/odd interleaves in one DVE read of x.
                # sum_c = 256*sum(means); sumsq_c = sum(cv) + 256*sum(mean^2)
                for ct in range(CT):
                    means = bn_sb[:, ct, b, 1::3]   # [P, 4] stride 3
                    cvs = bn_sb[:, ct, b, 2::3]     # [P, 4]
                    nc.vector.reduce_sum(
                        out=msum_c[:, ct, b:b + 1], in_=means, axis=AX)
                    nc.vector.reduce_sum(
                        out=cv_c[:, ct, b:b + 1], in_=cvs, axis=AX)
                    jt = junk_pool.tile([P, 4], f32, tag="junk4")
                    msq = junk_pool.tile([P, 1], f32, tag="junk1")
                    nc.vector.affine_mul_reduce(
                        out=jt, accum_out=msq, in0=means, in1=means,
                        scale=1.0, bias=0.0)
                    nc.vector.affine_then_add(
                        out=s2_c[:, ct, b:b + 1], in0=msq,
                        in1=cv_c[:, ct, b:b + 1], scale=256.0, bias=0.0)

            def emit_gn(b, xn_engines):
                # per-sample GroupNorm coefficients (a, bb) + xn write.
                # gm columns are host-prescaled: ps_g = [-mean | E[x^2]]
                ps_g = psum_small.tile([16, 4], f32, tag="pss")
                for ct in range(CT):
                    nc.tensor.matmul(ps_g[:, ct:ct + 1], gm_sb[:, 0:16],
                                     msum_c[:, ct, b:b + 1],
                                     start=True, stop=True)
                    nc.tensor.matmul(ps_g[:, 2 + ct:3 + ct], gm_sb[:, 16:32],
                                     s2_c[:, ct, b:b + 1],
                                     start=True, stop=True)
                nmean = persist.tile([16, 2], f32)
                nc.vector.tensor_copy(out=nmean, in_=ps_g[:, 0:2])
                msq = persist.tile([16, 2], f32)
                nc.vector.tensor_mul(msq, nmean, nmean)
                vpe = persist.tile([16, 2], f32)
                nc.vector.tensor_sub(vpe, ps_g[:, 2:4], msq)
                nc.vector.tensor_scalar_add(vpe, vpe, EPS)
                # rstd = 1/sqrt(var+eps): GN var ~1 so z0 = 1.5-(var+eps)/2
                # is within 0.4%; one Newton step reaches ~2e-5.
                rsm = persist.tile([16, 4], f32)
                z = rsm[:, 0:2]
                nc.vector.tensor_scalar(out=z, in0=vpe, scalar1=-0.5,
                                        scalar2=1.5, op0=ALU.mult, op1=ALU.add)
                zt = persist.tile([16, 2], f32)
                nc.vector.tensor_mul(zt, z, z)
                nc.vector.tensor_mul(zt, zt, vpe)
                nc.vector.tensor_scalar(out=zt, in0=zt, scalar1=-0.5,
                                        scalar2=1.5, op0=ALU.mult,
                                        op1=ALU.add)
                nc.vector.tensor_mul(z, z, zt)
                nc.vector.tensor_mul(rsm[:, 2:4], nmean, z)
                ps_bc = psum_small.tile([P, 4], f32, tag="pss")
                nc.tensor.matmul(ps_bc, gmt_sb, rsm, start=True, stop=True)
                if has_gn:
                    for ct in range(CT):
                        nc.vector.tensor_scalar_mul(
                            ab_sb[:, b, ct:ct + 1], ps_bc[:, ct:ct + 1],
                            gamma_sb[:, ct:ct + 1])
                        nc.vector.tensor_scalar(
                            out=ab_sb[:, b, 2 + ct:3 + ct],
                            in0=ps_bc[:, 2 + ct:3 + ct],
                            scalar1=gamma_sb[:, ct:ct + 1],
                            scalar2=beta_sb[:, ct:ct + 1],
                            op0=ALU.mult, op1=ALU.add)
                else:
                    nc.vector.tensor_copy(out=ab_sb[:, b], in_=ps_bc)
                for ct in range(CT):
                    eng = xn_engines[ct]
                    a_col = ab_sb[:, b, ct:ct + 1]
                    b_col = ab_sb[:, b, 2 + ct:3 + ct]
                    if eng == "act":
                        nc.scalar.activation(
                            out=xn_sb[:, ct, b], in_=x_sb[:, ct, b],
                            func=AF.Identity, bias=b_col, scale=a_col)
                    elif eng == "dve":
                        nc.vector.tensor_scalar(
                            out=xn_sb[:, ct, b], in0=x_sb[:, ct, b],
                            scalar1=a_col, scalar2=b_col,
                            op0=ALU.mult, op1=ALU.add)
                    else:
                        nc.gpsimd.tensor_scalar(
                            out=xn_sb[:, ct, b], in0=x_sb[:, ct, b],
                            scalar1=a_col, scalar2=b_col,
                            op0=ALU.mult, op1=ALU.add)

            def emit_se(b):
                # sigmoid(z)/1024 = 1/(1024*(1+exp(-z))): stays in exp table
                # and folds the fp8 weight-scale compensation in for free.
                ps_h1 = psum_small.tile([64, 1], f32, tag="pss")
                for ct in range(CT):
                    nc.tensor.matmul(ps_h1, w1_sb[:, ct],
                                     msum_c[:, ct, b:b + 1],
                                     start=(ct == 0), stop=(ct == 1))
                # pooled = msum*256/1024 -> relu scale 0.25
                nc.scalar.activation(out=h1_sb[:, b:b + 1], in_=ps_h1,
                                     func=AF.Relu, bias=b1_sb[:, 0:1],
                                     scale=0.25)
                for ot in range(CT):
                    ps_gate = psum_small.tile([P, 1], f32, tag="pss")
                    nc.tensor.matmul(ps_gate, w2_sb[:, ot * P:(ot + 1) * P],
                                     h1_sb[:, b:b + 1], start=True, stop=True)
                    eg = persist.tile([P, 1], f32)
                    nc.scalar.activation(out=eg, in_=ps_gate, func=AF.Exp,
                                         scale=-1.0, bias=nb2_sb[:, ot:ot + 1])
                    nc.vector.tensor_scalar(
                        out=eg, in0=eg, scalar1=1024.0, scalar2=1024.0,
                        op0=ALU.mult, op1=ALU.add)
                    nc.vector.reciprocal(gate_sb[:, ot, b:b + 1], eg)
                    if has_bp:
                        # bp_sb holds 1024*bp -> bpg = bp*sigmoid
                        nc.vector.tensor_scalar_mul(bpg_sb[:, ot, b:b + 1],
                                                    gate_sb[:, ot, b:b + 1],
                                                    bp_sb[:, ot:ot + 1])

            def emit_u(b, both_act=False):
                # zero-bias path: u = (G/GS) @ xn so that S = xn^T u.
                # Two fp8 tiles replace four (q0,q1,k0,k1) evacuations;
                # copies split ACT/DVE to balance the two evac engines.
                u_sb = u_pool.tile([P, 2, N], fp8, tag="uu")
                qk_tiles[b] = u_sb
                for at in range(CT):
                    ps_u = psum_big.tile([P, N], f32, tag="psb")
                    for ns in range(2):
                        nc.tensor.matmul(
                            ps_u[:, ns * 512:(ns + 1) * 512],
                            gt_sb[:, :, at * P:(at + 1) * P],
                            xn_sb[:, :, b, ns * 512:(ns + 1) * 512],
                            start=True, stop=True, perf_mode=DR)
                    if at == 0 or both_act:
                        nc.scalar.copy(out=u_sb[:, at], in_=ps_u)
                    else:
                        nc.vector.tensor_copy(out=u_sb[:, at], in_=ps_u)

            def emit_qk_biased(b):
                # general path with nonzero qkv bias: q, k : [c, n] fp8
                qk_sb = u_pool.tile([P, 4, N], fp8, tag="uu")
                qk_tiles[b] = qk_sb
                for m in (2, 3, 0, 1):
                    ps_qk = psum_big.tile([P, N], f32, tag="psb")
                    for ns in range(2):
                        nc.tensor.matmul(
                            ps_qk[:, ns * 512:(ns + 1) * 512],
                            wqk_sb[:, :, m * P:(m + 1) * P],
                            xn_sb[:, :, b, ns * 512:(ns + 1) * 512],
                            start=True, stop=True, perf_mode=DR)
                    nc.scalar.activation(out=qk_sb[:, m], in_=ps_qk,
                                         func=AF.Identity,
                                         bias=bqk_sb[:, m:m + 1])

            def emit_vt_mms(b, jps, ps_vts, vt_sb):
                for jp in jps:
                    ps_vt = psum_small.tile([P, 2, C], f32, tag="pss")
                    ps_vts[jp] = ps_vt
                    for j2 in range(2):
                        jt = 2 * jp + j2
                        nc.tensor.matmul(
                            ps_vt[:, j2],
                            xn_sb[:, :, b, jt * P:(jt + 1) * P],
                            wv_sb[:, :, :],
                            start=True, stop=True, perf_mode=DR)
                    # vt evacuates on DVE: keeps the ACT queue clear for exps
                    nc.vector.tensor_copy(
                        out=vt_sb[:, 2 * jp:2 * jp + 2], in_=ps_vt)

            def emit_s_part(b, es_sb, mts):
                src = qk_tiles[b]
                for mt in mts:
                    ps_s = psum_big.tile([P, N], f32, tag="psb")
                    for ns in range(2):
                        if has_bqk:
                            nc.tensor.matmul(
                                ps_s[:, ns * 512:(ns + 1) * 512],
                                src[:, 2:4, mt * P:(mt + 1) * P],
                                src[:, 0:2, ns * 512:(ns + 1) * 512],
                                start=True, stop=True, perf_mode=DR)
                        else:
                            nc.tensor.matmul(
                                ps_s[:, ns * 512:(ns + 1) * 512],
                                xn_sb[:, :, b, mt * P:(mt + 1) * P],
                                src[:, :, ns * 512:(ns + 1) * 512],
                                start=True, stop=True, perf_mode=DR)
                    nc.scalar.activation(out=es_sb[:, mt], in_=ps_s,
                                         func=AF.Exp, scale=ES_SCALE)

            def emit_sums_av(b, vt_sb, es_sb):
                # sums accumulate in two 1-bank pss halves so the big pool's
                # 12-slot rotation (mt0..7, u0, u1, av0, av1) stays stable:
                # u gets mid-stream-freed buffers (exp5/exp6) and only av
                # couples to the end of the exp stream.
                ps_sums = [psum_small.tile([P, 512], f32, tag="pss",
                                           name=f"ps_sum{_h}")
                           for _h in range(2)]
                xat_sb = xat_pool.tile([P, CT, N], fp8, tag="xat")
                for jp in range(4):
                    jsl = slice(2 * jp, 2 * jp + 2)
                    for ns in range(2):
                        hs = slice(ns * 512, (ns + 1) * 512)
                        nc.tensor.matmul(
                            ps_sums[ns], ones_sb, es_sb[:, jsl, hs],
                            start=(jp == 0), stop=(jp == 3), perf_mode=DR)
                ps_avs = [psum_big.tile([P, N], f32, tag="psb",
                                        name=f"ps_av{_i}") for _i in range(CT)]
                for ct2 in range(CT):
                    for jp in range(4):
                        jsl = slice(2 * jp, 2 * jp + 2)
                        for ns in range(2):
                            hs = slice(ns * 512, (ns + 1) * 512)
                            nc.tensor.matmul(
                                ps_avs[ct2][:, hs],
                                vt_sb[:, jsl, ct2 * P:(ct2 + 1) * P],
                                es_sb[:, jsl, hs],
                                start=(jp == 0), stop=(jp == 3), perf_mode=DR)
                r_sb = r_pool.tile([P, N], f32, tag="rr")
                for h in range(2):
                    hs = slice(h * 512, (h + 1) * 512)
                    nc.vector.reciprocal_approx_fast(out=r_sb[:, hs],
                                                     in_=ps_sums[h])
                # ct0 first, per half: frees av0's buffer earliest (the next
                # sample's S mt1 waits on it)
                for ct2 in range(CT):
                    for h in range(2):
                        hs = slice(h * 512, (h + 1) * 512)
                        if has_bv:
                            tmp = r_pool.tile([P, N], f32, tag="avtmp")
                            nc.vector.tensor_mul(tmp[:, hs],
                                                 ps_avs[ct2][:, hs],
                                                 r_sb[:, hs])
                            nc.vector.tensor_scalar_add(
                                xat_sb[:, ct2, hs], tmp[:, hs],
                                bv_sb[:, ct2:ct2 + 1])
                        else:
                            nc.vector.tensor_mul(xat_sb[:, ct2, hs],
                                                 ps_avs[ct2][:, hs],
                                                 r_sb[:, hs])
                return xat_sb

            def emit_proj_fuse(b, xat_sb):
                # proj runs in 1-bank pss quarters (keeps the big pool's
                # rotation to the S/u/av tiles); each quarter evacuates with
                # one fused DVE (y*gate + x) pass and DMAs immediately.
                out_ts = [out_pool.tile([P, N], f32, tag="outp",
                                        name=f"out_t{_i}") for _i in range(CT)]
                for h in range(2):
                    hs = slice(h * 512, (h + 1) * 512)
                    for ot in range(CT):
                        ps_yq = psum_small.tile([P, 512], f32, tag="pss",
                                                name=f"ps_y{ot}_{h}")
                        nc.tensor.matmul(
                            ps_yq,
                            wp_sb[:, :, ot * P:(ot + 1) * P],
                            xat_sb[:, :, hs],
                            start=True, stop=True, perf_mode=DR)
                        out_t = out_ts[ot]
                        nc.vector.affine_then_add(
                            out=out_t[:, hs], in0=ps_yq,
                            in1=x_sb[:, ot, b, hs],
                            scale=gate_sb[:, ot, b:b + 1], bias=0.0)
                        if has_bp:
                            nc.vector.tensor_scalar_add(
                                out_t[:, hs], out_t[:, hs],
                                bpg_sb[:, ot, b:b + 1])
                        nc.sync.dma_start(
                            out=out_d[b, ot * P:(ot + 1) * P, hs],
                            in_=out_t[:, hs])

            ES_SCALE = 0.0625 / (WS * WS) if has_bqk else 0.0625 / GS

            # ---- PE warm-up: dead matmuls during the DMA/stats head so
            # the HAM clock-gate reaches 8/8 before real matmuls ----
            def warm(nmm):
                ps_warm = psum_big.tile([P, 512], f32, tag="psb",
                                        name="ps_warm")
                for _ in range(nmm):
                    nc.tensor.matmul(ps_warm, warm_sb[:, 0:P], warm_sb,
                                     start=True, stop=True)

            def emit_first(b, both_act=False):
                if has_bqk:
                    emit_qk_biased(b)
                else:
                    emit_u(b, both_act=both_act)

            # ---- schedule: all stats/GN/SE work lives in the head or the
            # slack of earlier samples; the loop's steady state is paced by
            # the ACT exp stream. proj(b) is software-pipelined into
            # iteration b+1 (between S mt1 and mt2) so the next sample's S
            # stream never stalls behind proj's wait on the DVE evacuations.
            warm(12)
            emit_stats_bn(0)
            emit_stats(0)
            emit_gn(0, xn_engines=("act", "dve"))
            emit_stats_bn(1)
            emit_stats(1)
            emit_first(0, both_act=True)
            emit_gn(1, xn_engines=("pool", "pool"))
            nc.vector.tensor_scalar_mul(nb2_sb, b2_sb, -1.0)
            emit_se(0)
            emit_se(1)
            pend_proj = None
            for b in range(BL):
                vt_sb = vt_pool.tile([P, 8, C], fp8, tag="vt")
                ps_vts = {}
                emit_vt_mms(b, (0, 1), ps_vts, vt_sb)
                es_sb = es_pool.tile([P, 8, N], fp8, tag="es")
                emit_s_part(b, es_sb, (0, 1))
                if pend_proj is not None:
                    emit_proj_fuse(*pend_proj)
                    pend_proj = None
                emit_s_part(b, es_sb, (2, 3))
                if b < 2:
                    # the bn passes for sample b+2 run mid-exp-stream so the
                    # later gn matmuls find their DVE inputs ready at the
                    # sample boundary instead of stalling the next S stream
                    emit_stats_bn(b + 2)
                if b in (1, 2):
                    # SE for sample b+1 was stats-prepped an iteration ago;
                    # emitting it here keeps its ACT ops (relu + 2 exps) out
                    # of the sample boundary where they blocked the next
                    # sample's exp stream for the whole upstream chain
                    emit_se(b + 1)
                emit_s_part(b, es_sb, (4, 5, 6, 7))
                emit_vt_mms(b, (2, 3), ps_vts, vt_sb)
                if b + 1 < BL:
                    emit_first(b + 1)
                else:
                    # keep the big pool's 12-slot rotation period on the last
                    # iteration (no u tiles): placeholder allocations only
                    psum_big.tile([P, N], f32, tag="psb", name="pad0")
                    psum_big.tile([P, N], f32, tag="psb", name="pad1")
                xat_sb = emit_sums_av(b, vt_sb, es_sb)
                pend_proj = (b, xat_sb)
                if b < 2:
                    emit_stats(b + 2)
                    emit_gn(b + 2, xn_engines=("pool", "pool"))
            emit_proj_fuse(*pend_proj)

    nc.compile()
    return nc


def _prep_inputs(x, gn_gamma, gn_beta, w_qkv, b_qkv, w_proj, b_proj,
                 w_se1, b_se1, w_se2, b_se2):
    fp8 = ml_dtypes.float8_e4m3
    f32 = np.float32

    def pt(w):  # [K, M] -> [128, K//128, M] partition-tiled
        K, M = w.shape
        return np.ascontiguousarray(w.reshape(K // P, P, M).transpose(1, 0, 2))

    wq = np.asarray(w_qkv[:C], np.float64)
    wk = np.asarray(w_qkv[C:2 * C], np.float64)
    # lhsT for u = G @ xn must be G^T[b, a] where G = Wk^T Wq, i.e. Wq^T Wk
    gt = (pt(wq.T @ wk) * GS).astype(fp8)
    wqk = (pt(np.ascontiguousarray(np.asarray(w_qkv[:512]).T)) * WS).astype(fp8)
    wv = (pt(np.ascontiguousarray(np.asarray(w_qkv[512:]).T)) * WS).astype(fp8)
    wp = (pt(np.ascontiguousarray(np.asarray(w_proj).T)) * WS).astype(fp8)
    w1 = pt(np.ascontiguousarray(np.asarray(w_se1).T)).astype(f32)
    w2 = np.ascontiguousarray(np.asarray(w_se2).T).astype(f32)

    def pcol(v):  # [256] -> [128, 2]
        return np.ascontiguousarray(np.asarray(v).reshape(2, P).T).astype(f32)

    gm1 = np.zeros((P, 16), f32)
    gm1[np.arange(P), np.arange(P) // GSIZE] = 1.0
    gm = np.concatenate([gm1 * (-256.0 / (GSIZE * N)),
                         gm1 * (1.0 / (GSIZE * N))], axis=1).astype(f32)
    shared = {
        "gt": gt, "wqk": wqk, "wv": wv, "wp": wp, "w1": w1, "w2": w2,
        "gamma": pcol(gn_gamma), "beta": pcol(gn_beta),
        # q/k biases land on 32x-scaled psums
        "bqk": np.ascontiguousarray(
            np.asarray(b_qkv[:512], f32).reshape(4, P).T) * WS,
        "bv": pcol(b_qkv[512:]) * WS,
        "bp": pcol(b_proj) * 1024.0,
        "b1": np.asarray(b_se1, f32).reshape(64, 1),
        "b2": pcol(b_se2),
        "gm": gm, "gmt": np.ascontiguousarray(gm1.T),
    }
    xr = np.asarray(x, f32).reshape(B, C, N)
    in_maps = []
    for i in range(NCORES):
        m = dict(shared)
        m["x"] = np.ascontiguousarray(xr[i * BL:(i + 1) * BL])
        in_maps.append(m)
    flags = (bool(np.any(np.asarray(b_qkv[:512]) != 0)),
             bool(np.any(np.asarray(b_qkv[512:]) != 0)),
             bool(np.any(np.asarray(b_proj) != 0)),
             bool(np.any(np.asarray(gn_gamma) != 1) or
                  np.any(np.asarray(gn_beta) != 0)))
    return in_maps, flags


def _get_program(flags):
    key = ("prog", flags)
    if key not in _CACHE:
        _CACHE[key] = _build_program(flags)
    return _CACHE[key]


def run(inputs, trace=False, trace_kwargs=None):
    """Build + run on all 8 cores. Returns (full_out, BassKernelResults)."""
    from concourse.bass_utils import run_bass_kernel_spmd

    in_maps, flags = _prep_inputs(**inputs)
    nc = _get_program(flags)
    kw = {}
    if trace:
        kw["trace"] = True
        if trace_kwargs:
            kw["trace_kwargs"] = trace_kwargs
    res = run_bass_kernel_spmd(nc, in_maps, list(range(NCORES)), **kw)
    out = np.concatenate([res.results[i]["out"] for i in range(NCORES)], axis=0)
    return out.reshape(B, C, HW, HW).astype(np.float32), res


def kernel(**inputs):
    out, _ = run(inputs, trace=False)
    return out
